# revision 16
# baseline (speedup 1.0000x reference)
"""Trainium2 Bass kernel for nn_MultiHeadAttention_71502615544564 (GNN
message-passing multi-head attention).

Math note (why this kernel is so small): the reference computes
    out_nodes = segment_sum(v[dst] * attn_weights[..., None], dst)
with attn_weights = exp_attn / (sum_exp[dst] + 1e-8).  Because v is
indexed by the SAME dst as the segment reduction, v[n] factors out of
each segment:
    out_nodes[n] = v[n] * (sum_e exp_attn[e]) / (sum_exp[n] + 1e-8)
                 = v[n] * s_n / (s_n + 1e-8).
For the given input regime the per-(node,head) softmax denominator s_n
is >= 2.6e-2 whenever node n has at least one incoming edge (attn
logits are O(1); verified on the actual inputs), so
    s_n / (s_n + 1e-8) = 1 - eps,   eps <= 4e-7,
and s_n = 0 (ratio 0) exactly when node n has no incoming edge.  The
entire q/k/exp/scatter pipeline therefore contributes only a <=4e-7
relative perturbation to the output:
    out[n] = deg_in(n) > 0 ? v[n] @ W_out + b_out : b_out.
Finally the two linear layers fold:  (x @ Wv + bv) @ W_out + b_out =
x @ (Wv @ W_out) + (bv @ W_out + b_out) = x @ W2 + b2, with W2/b2
folded once on the host (weights-only constant folding).

The device kernel computes out = x @ W2 + b2 for its node shard
(nodes are sharded 8 ways, 6250 per core; x shard is staged
transposed so the contraction dim is the partition dim).  bf16
inputs / f32 PSUM accumulation give ~2.4e-3 max relative error
(gate: 2e-2).  Zero-in-degree nodes (none in the actual inputs) are
patched to b_out during unsharding.
"""

import sys

sys.path.insert(0, "/opt/trn_rl_repo")

import ml_dtypes
import numpy as np

import concourse.bacc as bacc
import concourse.mybir as mybir
import concourse.tile as tile
from concourse.bass_utils import run_bass_kernel_spmd

P = 128
N, DIM, H, HD = 50000, 128, 8, 16
NCORES = 8
NLOC = N // NCORES            # 6250 nodes per core
NKC = (NLOC + P - 1) // P     # 49 column tiles
NKR = NKC * P                 # 6272 padded columns
CH = 512                      # matmul chunk (one PSUM bank of f32)

F32 = mybir.dt.float32
BF16 = mybir.dt.bfloat16
BF = ml_dtypes.bfloat16


def build_program():
    nc = bacc.Bacc("TRN2", target_bir_lowering=False, debug=False)

    xlT = nc.dram_tensor("xlT", [P, NKR], BF16, kind="ExternalInput")
    w2 = nc.dram_tensor("w2", [DIM, DIM], BF16, kind="ExternalInput")
    b2 = nc.dram_tensor("b2", [DIM, 1], F32, kind="ExternalInput")
    outT = nc.dram_tensor("outT", [P, NKR], BF16, kind="ExternalOutput")

    NCH = (NKR + CH - 1) // CH  # 13 chunks
    with tile.TileContext(nc) as tc:
        with (
            tc.tile_pool(name="const", bufs=1) as cpool,
            tc.tile_pool(name="xs", bufs=1) as xpool,
            tc.tile_pool(name="os", bufs=1) as opool,
            tc.tile_pool(name="ps", bufs=8, space="PSUM") as psum,
        ):
            w2_sb = cpool.tile([DIM, DIM], BF16)
            b2_sb = cpool.tile([DIM, 1], F32)

            # Per-slice tiles so each consumer depends only on its own
            # DMA (dependency tracking is whole-tile).  DMA triggers cost
            # ~600ns of issuing-engine time each, so slices are coarse
            # (1024 cols) and spread over the three DMA-capable engine
            # queues (gpsimd/sync/scalar).  The kernel is HBM-bound:
            # 1.6 MB in + 1.6 MB out at ~358 GB/s/core sets the floor.
            # First input slice is small so the matmul pipeline starts
            # early; matmuls stay 512 wide (one f32 PSUM bank).
            nc.scalar.dma_start(out=w2_sb[:], in_=w2[:])
            # 128-col first slice so the matmul pipeline ramps up fast
            widths = [128] + [CH] * 12  # sum = NKR
            cuts = [0]
            for w in widths:
                cuts.append(cuts[-1] + w)
            assert cuts[-1] == NKR
            NS = len(cuts) - 1
            xts = []
            for k in range(NS):
                b0, b1 = cuts[k], cuts[k + 1]
                xt = xpool.tile([P, b1 - b0], BF16, tag=f"xt{k}")
                eng = nc.gpsimd if k % 2 == 0 else nc.sync
                eng.dma_start(out=xt[:], in_=xlT[:, b0:b1])
                xts.append(xt)
            nc.scalar.dma_start(out=b2_sb[:], in_=b2[:])

            # adds: vector mostly, scalar for every third chunk and the
            # last one; a scalar-added chunk issues its own out-DMA
            # (scalar is DMA-capable), skipping a cross-engine semaphore
            # hop — and ending the pipeline on such a chunk shortens the
            # drain tail
            scalar_adds = {2, 5, 8, 12}
            for k in range(NS):
                b0, b1 = cuts[k], cuts[k + 1]
                nb = b1 - b0
                ps = psum.tile([P, CH], F32, tag="ps")
                nc.tensor.matmul(out=ps[:, :nb], lhsT=w2_sb[:],
                                 rhs=xts[k][:], start=True, stop=True)
                ot = opool.tile([P, nb], BF16, tag=f"ot{k}")
                # psum + b2 (per-partition bias); only vector/scalar can
                # read PSUM
                if k in scalar_adds:
                    nc.scalar.activation(
                        out=ot[:], in_=ps[:, :nb],
                        func=mybir.ActivationFunctionType.Identity,
                        bias=b2_sb[:], scale=1.0)
                    nc.scalar.dma_start(out=outT[:, b0:b1], in_=ot[:])
                else:
                    nc.vector.tensor_scalar_add(out=ot[:], in0=ps[:, :nb],
                                                scalar1=b2_sb[:])
                    eng = nc.sync if k % 2 == 0 else nc.gpsimd
                    eng.dma_start(out=outT[:, b0:b1], in_=ot[:])

    nc.compile()
    return nc


def _prep(x, edge_index, W_qkv, b_qkv, W_out, b_out):
    x = np.asarray(x, np.float32)
    ei = np.asarray(edge_index, np.int64)
    W_qkv = np.asarray(W_qkv, np.float32)
    b_qkv = np.asarray(b_qkv, np.float32)
    W_out = np.asarray(W_out, np.float32)
    b_out = np.asarray(b_out, np.float32)

    # v columns of the packed qkv projection: head h occupies columns
    # [h*3*HD, (h+1)*3*HD) with v in the last HD of each group
    hh = np.arange(H)[:, None]
    dd = np.arange(HD)[None, :]
    cols_v = (hh * 3 * HD + 2 * HD + dd).ravel()

    # fold the two linear layers (weights-only constant folding)
    Wv = W_qkv[:, cols_v].astype(np.float64)
    bv = b_qkv[cols_v].astype(np.float64)
    W2 = (Wv @ W_out.astype(np.float64)).astype(np.float32)
    b2 = (bv @ W_out.astype(np.float64) + b_out).astype(np.float32)

    common = {
        "w2": W2.astype(BF),
        "b2": b2.reshape(DIM, 1).copy(),
    }
    in_maps = []
    for c in range(NCORES):
        xl = np.zeros((P, NKR), BF)
        xl[:, :NLOC] = x[c * NLOC:(c + 1) * NLOC].T.astype(BF)
        in_maps.append({**common, "xlT": xl})

    # nodes with no incoming edge get b_out exactly (none in practice)
    deg = np.bincount(ei[1], minlength=N)
    zero_deg = np.where(deg == 0)[0]
    return in_maps, zero_deg, b_out


_PROG_CACHE = {}
TRACE = False
LAST_RESULT = None


def _install_ntff_hook():
    """Provide antenv.axon_hooks (absent in this image) so
    run_bass_kernel_spmd(trace=True) can NTFF-profile via libaxon."""
    import contextlib
    import ctypes
    import types

    if "antenv.axon_hooks" in sys.modules:
        return
    try:
        from antenv import axon_hooks  # noqa: F401
        return
    except ImportError:
        pass
    so_path = "/opt/axon/libaxon_pjrt.so"
    try:
        lib = ctypes.CDLL(so_path)
    except OSError:
        return
    if not hasattr(lib, "axon_start_nrt_profile"):
        return
    lib.axon_start_nrt_profile.argtypes = [
        ctypes.POINTER(ctypes.c_int64), ctypes.c_size_t]
    lib.axon_start_nrt_profile.restype = ctypes.c_int64
    lib.axon_stop_nrt_profile.argtypes = [ctypes.c_char_p]
    lib.axon_stop_nrt_profile.restype = ctypes.c_int64

    @contextlib.contextmanager
    def _hook(output_dir, device_ids):
        import jax
        jax.devices()
        if device_ids:
            ids = (ctypes.c_int64 * len(device_ids))(*device_ids)
            rc = lib.axon_start_nrt_profile(ids, len(device_ids))
        else:
            rc = lib.axon_start_nrt_profile(None, 0)
        if rc != 0:
            raise RuntimeError(f"axon_start_nrt_profile rc={rc}")
        try:
            yield
        finally:
            n = lib.axon_stop_nrt_profile(str(output_dir).encode())
            print(f"ntff profile: {n} file(s) -> {output_dir}", file=sys.stderr)

    _h = [_hook]
    m = types.ModuleType("antenv.axon_hooks")
    m.get_axon_ntff_profile_hook = lambda: _h[0]
    m.set_axon_ntff_profile_hook = lambda h: _h.__setitem__(0, h)
    sys.modules["antenv.axon_hooks"] = m
    import antenv
    antenv.axon_hooks = m


def kernel(x, edge_index, W_qkv, b_qkv, W_out, b_out):
    in_maps, zero_deg, b_out_f = _prep(x, edge_index, W_qkv, b_qkv,
                                       W_out, b_out)
    if "prog" not in _PROG_CACHE:
        _PROG_CACHE["prog"] = build_program()
    nc = _PROG_CACHE["prog"]
    if TRACE:
        _install_ntff_hook()
    res = run_bass_kernel_spmd(nc, in_maps, list(range(NCORES)), trace=TRACE)
    global LAST_RESULT
    LAST_RESULT = res
    out = np.empty((N, DIM), np.float32)
    for c in range(NCORES):
        o = np.asarray(res.results[c]["outT"])
        out[c * NLOC:(c + 1) * NLOC] = o[:, :NLOC].T
    if len(zero_deg):
        out[zero_deg] = b_out_f
    return out


if __name__ == "__main__":
    rng = np.random.default_rng(0)
    x = rng.standard_normal((N, DIM)).astype(np.float32)
    ei = rng.integers(0, N, (2, 640000)).astype(np.int64)
    lim = 1.0 / np.sqrt(DIM)
    W_qkv = rng.uniform(-lim, lim, (DIM, 3 * DIM)).astype(np.float32)
    b_qkv = rng.uniform(-lim, lim, (3 * DIM,)).astype(np.float32)
    W_out = rng.uniform(-lim, lim, (DIM, DIM)).astype(np.float32)
    b_out = rng.uniform(-lim, lim, (DIM,)).astype(np.float32)
    out = kernel(x=x, edge_index=ei, W_qkv=W_qkv, b_qkv=b_qkv,
                 W_out=W_out, b_out=b_out)
    print("kernel output:", out.shape, out.dtype, np.abs(out).max())


# revision 19
# speedup vs baseline: 1.1433x; 1.1433x over previous
"""Trainium2 Bass kernel for nn_MultiHeadAttention_71502615544564 (GNN
message-passing multi-head attention).

Math note (why this kernel is so small): the reference computes
    out_nodes = segment_sum(v[dst] * attn_weights[..., None], dst)
with attn_weights = exp_attn / (sum_exp[dst] + 1e-8).  Because v is
indexed by the SAME dst as the segment reduction, v[n] factors out of
each segment:
    out_nodes[n] = v[n] * (sum_e exp_attn[e]) / (sum_exp[n] + 1e-8)
                 = v[n] * s_n / (s_n + 1e-8).
For the given input regime the per-(node,head) softmax denominator s_n
is >= 2.6e-2 whenever node n has at least one incoming edge (attn
logits are O(1); verified on the actual inputs), so
    s_n / (s_n + 1e-8) = 1 - eps,   eps <= 4e-7,
and s_n = 0 (ratio 0) exactly when node n has no incoming edge.  The
entire q/k/exp/scatter pipeline therefore contributes only a <=4e-7
relative perturbation to the output:
    out[n] = deg_in(n) > 0 ? v[n] @ W_out + b_out : b_out.
Finally the two linear layers fold:  (x @ Wv + bv) @ W_out + b_out =
x @ (Wv @ W_out) + (bv @ W_out + b_out) = x @ W2 + b2, with W2/b2
folded once on the host (weights-only constant folding).

The device kernel computes out = x @ W2 + b2 for its node shard
(nodes are sharded 8 ways, 6250 per core; x shard is staged
transposed so the contraction dim is the partition dim).  bf16
inputs / f32 PSUM accumulation / bf16 output give ~3.8e-3 max
relative error (gate: 2e-2).  Zero-in-degree nodes (none in the
actual inputs) are patched to b_out during unsharding.

Performance: the kernel is HBM-bound — 1.6 MB in + 1.6 MB out per
core at ~358 GB/s/core is ~9 us; NEFF start/stop overhead is ~12 us
(measured with a trivial kernel); measured exec is ~24 us (baseline:
2753 us).  Work is sliced 512 cols at a time (one f32 PSUM bank),
with per-slice SBUF tiles so each pipeline stage depends only on its
own slice, and DMA triggers (~600ns of issuing-engine time each)
spread across the three DMA-capable queues (sync/gpsimd/scalar).
"""

import sys

sys.path.insert(0, "/opt/trn_rl_repo")

import ml_dtypes
import numpy as np

import concourse.bacc as bacc
import concourse.mybir as mybir
import concourse.tile as tile
from concourse.bass_utils import run_bass_kernel_spmd

P = 128
N, DIM, H, HD = 50000, 128, 8, 16
NCORES = 8
NLOC = N // NCORES            # 6250 nodes per core
NKC = (NLOC + P - 1) // P     # 49 column tiles
NKR = NKC * P                 # 6272 padded columns
CH = 512                      # matmul chunk (one PSUM bank of f32)

F32 = mybir.dt.float32
BF16 = mybir.dt.bfloat16
BF = ml_dtypes.bfloat16


def build_program():
    nc = bacc.Bacc("TRN2", target_bir_lowering=False, debug=False)

    xlT = nc.dram_tensor("xlT", [P, NKR], BF16, kind="ExternalInput")
    w2 = nc.dram_tensor("w2", [DIM, DIM], BF16, kind="ExternalInput")
    b2 = nc.dram_tensor("b2", [DIM, 1], F32, kind="ExternalInput")
    outT = nc.dram_tensor("outT", [P, NKR], BF16, kind="ExternalOutput")

    with tile.TileContext(nc) as tc:
        with (
            tc.tile_pool(name="const", bufs=1) as cpool,
            tc.tile_pool(name="xs", bufs=1) as xpool,
            tc.tile_pool(name="os", bufs=1) as opool,
            tc.tile_pool(name="ps", bufs=8, space="PSUM") as psum,
        ):
            w2_sb = cpool.tile([DIM, DIM], BF16)
            b2_sb = cpool.tile([DIM, 1], F32)

            # Per-slice tiles so each consumer depends only on its own
            # DMA (dependency tracking is whole-tile).  DMA triggers cost
            # ~600ns of issuing-engine time each, so input issues
            # alternate between the gpsimd and sync queues while the
            # scalar queue loads the constants.  A small 128-col first
            # slice lets the matmul pipeline ramp up early.
            nc.scalar.dma_start(out=w2_sb[:], in_=w2[:])
            widths = [128] + [CH] * 12  # sum = NKR
            cuts = [0]
            for w in widths:
                cuts.append(cuts[-1] + w)
            assert cuts[-1] == NKR
            NS = len(cuts) - 1
            xts = []
            for k in range(NS):
                b0, b1 = cuts[k], cuts[k + 1]
                xt = xpool.tile([P, b1 - b0], BF16, tag=f"xt{k}")
                eng = nc.gpsimd if k % 2 == 0 else nc.sync
                eng.dma_start(out=xt[:], in_=xlT[:, b0:b1])
                xts.append(xt)
            nc.scalar.dma_start(out=b2_sb[:], in_=b2[:])

            # adds: vector mostly, scalar for every third chunk and the
            # last one; a scalar-added chunk issues its own out-DMA
            # (scalar is DMA-capable), skipping a cross-engine semaphore
            # hop — and ending the pipeline on such a chunk shortens the
            # drain tail
            scalar_adds = {2, 5, 8, 12}
            for k in range(NS):
                b0, b1 = cuts[k], cuts[k + 1]
                nb = b1 - b0
                ps = psum.tile([P, CH], F32, tag="ps")
                nc.tensor.matmul(out=ps[:, :nb], lhsT=w2_sb[:],
                                 rhs=xts[k][:], start=True, stop=True)
                ot = opool.tile([P, nb], BF16, tag=f"ot{k}")
                # psum + b2 (per-partition bias); only vector/scalar can
                # read PSUM
                if k in scalar_adds:
                    nc.scalar.activation(
                        out=ot[:], in_=ps[:, :nb],
                        func=mybir.ActivationFunctionType.Identity,
                        bias=b2_sb[:], scale=1.0)
                    nc.scalar.dma_start(out=outT[:, b0:b1], in_=ot[:])
                else:
                    nc.vector.tensor_scalar_add(out=ot[:], in0=ps[:, :nb],
                                                scalar1=b2_sb[:])
                    eng = nc.sync if k % 2 == 0 else nc.gpsimd
                    eng.dma_start(out=outT[:, b0:b1], in_=ot[:])

    nc.compile()
    return nc


def _prep(x, edge_index, W_qkv, b_qkv, W_out, b_out):
    x = np.asarray(x, np.float32)
    ei = np.asarray(edge_index, np.int64)
    W_qkv = np.asarray(W_qkv, np.float32)
    b_qkv = np.asarray(b_qkv, np.float32)
    W_out = np.asarray(W_out, np.float32)
    b_out = np.asarray(b_out, np.float32)

    # v columns of the packed qkv projection: head h occupies columns
    # [h*3*HD, (h+1)*3*HD) with v in the last HD of each group
    hh = np.arange(H)[:, None]
    dd = np.arange(HD)[None, :]
    cols_v = (hh * 3 * HD + 2 * HD + dd).ravel()

    # fold the two linear layers (weights-only constant folding)
    Wv = W_qkv[:, cols_v].astype(np.float64)
    bv = b_qkv[cols_v].astype(np.float64)
    W2 = (Wv @ W_out.astype(np.float64)).astype(np.float32)
    b2 = (bv @ W_out.astype(np.float64) + b_out).astype(np.float32)

    common = {
        "w2": W2.astype(BF),
        "b2": b2.reshape(DIM, 1).copy(),
    }
    in_maps = []
    for c in range(NCORES):
        xl = np.zeros((P, NKR), BF)
        xl[:, :NLOC] = x[c * NLOC:(c + 1) * NLOC].T.astype(BF)
        in_maps.append({**common, "xlT": xl})

    # nodes with no incoming edge get b_out exactly (none in practice)
    deg = np.bincount(ei[1], minlength=N)
    zero_deg = np.where(deg == 0)[0]
    return in_maps, zero_deg, b_out


_PROG_CACHE = {}
TRACE = False
LAST_RESULT = None


def _install_ntff_hook():
    """Provide antenv.axon_hooks (absent in this image) so
    run_bass_kernel_spmd(trace=True) can NTFF-profile via libaxon."""
    import contextlib
    import ctypes
    import types

    if "antenv.axon_hooks" in sys.modules:
        return
    try:
        from antenv import axon_hooks  # noqa: F401
        return
    except ImportError:
        pass
    so_path = "/opt/axon/libaxon_pjrt.so"
    try:
        lib = ctypes.CDLL(so_path)
    except OSError:
        return
    if not hasattr(lib, "axon_start_nrt_profile"):
        return
    lib.axon_start_nrt_profile.argtypes = [
        ctypes.POINTER(ctypes.c_int64), ctypes.c_size_t]
    lib.axon_start_nrt_profile.restype = ctypes.c_int64
    lib.axon_stop_nrt_profile.argtypes = [ctypes.c_char_p]
    lib.axon_stop_nrt_profile.restype = ctypes.c_int64

    @contextlib.contextmanager
    def _hook(output_dir, device_ids):
        import jax
        jax.devices()
        if device_ids:
            ids = (ctypes.c_int64 * len(device_ids))(*device_ids)
            rc = lib.axon_start_nrt_profile(ids, len(device_ids))
        else:
            rc = lib.axon_start_nrt_profile(None, 0)
        if rc != 0:
            raise RuntimeError(f"axon_start_nrt_profile rc={rc}")
        try:
            yield
        finally:
            n = lib.axon_stop_nrt_profile(str(output_dir).encode())
            print(f"ntff profile: {n} file(s) -> {output_dir}", file=sys.stderr)

    _h = [_hook]
    m = types.ModuleType("antenv.axon_hooks")
    m.get_axon_ntff_profile_hook = lambda: _h[0]
    m.set_axon_ntff_profile_hook = lambda h: _h.__setitem__(0, h)
    sys.modules["antenv.axon_hooks"] = m
    import antenv
    antenv.axon_hooks = m


def kernel(x, edge_index, W_qkv, b_qkv, W_out, b_out):
    in_maps, zero_deg, b_out_f = _prep(x, edge_index, W_qkv, b_qkv,
                                       W_out, b_out)
    if "prog" not in _PROG_CACHE:
        _PROG_CACHE["prog"] = build_program()
    nc = _PROG_CACHE["prog"]
    if TRACE:
        _install_ntff_hook()
    res = run_bass_kernel_spmd(nc, in_maps, list(range(NCORES)), trace=TRACE)
    global LAST_RESULT
    LAST_RESULT = res
    out = np.empty((N, DIM), np.float32)
    for c in range(NCORES):
        o = np.asarray(res.results[c]["outT"])
        out[c * NLOC:(c + 1) * NLOC] = o[:, :NLOC].T
    if len(zero_deg):
        out[zero_deg] = b_out_f
    return out


if __name__ == "__main__":
    rng = np.random.default_rng(0)
    x = rng.standard_normal((N, DIM)).astype(np.float32)
    ei = rng.integers(0, N, (2, 640000)).astype(np.int64)
    lim = 1.0 / np.sqrt(DIM)
    W_qkv = rng.uniform(-lim, lim, (DIM, 3 * DIM)).astype(np.float32)
    b_qkv = rng.uniform(-lim, lim, (3 * DIM,)).astype(np.float32)
    W_out = rng.uniform(-lim, lim, (DIM, DIM)).astype(np.float32)
    b_out = rng.uniform(-lim, lim, (DIM,)).astype(np.float32)
    out = kernel(x=x, edge_index=ei, W_qkv=W_qkv, b_qkv=b_qkv,
                 W_out=W_out, b_out=b_out)
    print("kernel output:", out.shape, out.dtype, np.abs(out).max())
